# revision 1
# baseline (speedup 1.0000x reference)
"""CATAttention Trainium2 kernel.

Math: out[b,i,h,:] = sum_{j<=i} softmax_s(x@W_A^T)[b,i-j,h] * v[b,j,h,:]
i.e. a causal convolution along the sequence with a per-(b,h) data-dependent
kernel z. The [B,H,S,S] "roll" matrix is block-Toeplitz: its 128x128 blocks
depend only on the block lag L = I-J, so only 16 distinct blocks per head are
ever materialized (built in SBUF by a sliding-window DMA from a zero-padded
copy of z in DRAM — the zero pad implements the causal mask for L=0).

Sharding (8 cores): core c -> batch b = c//4, head group g = c%4 (4 heads).
Each core computes z, v = x@W_V^T (its 256 channels), the causal Toeplitz
matmul, and a partial output projection against its 256 columns of W_O.
Host gathers: out[b] = sum of the 4 partials + b_O.

All matmuls run as float32r (fp32 data, single-pass PE mode) which streams at
bf16 rate for moving dims >= 256.
"""

import numpy as np

import concourse.bass as bass
import concourse.mybir as mybir
import concourse.tile as tile
from concourse import masks
from concourse.ap import AP

F32 = mybir.dt.float32
F32R = mybir.dt.float32r
BF16 = mybir.dt.bfloat16
OUT_BF16 = True

B, S, E, H, D = 2, 2048, 1024, 16, 64
SCALING = D ** -0.5
NCORES = 8
HPC = 4            # heads per core
CB = HPC * D       # 256 channels per core
NB = S // 128      # 16 seq blocks
KE = E // 128      # 8 contraction chunks
ZW = 2176          # zpad row width: 128 zeros + 2048 weights


def _split_excess_waits(nc, max_waits=1):
    """The walrus in this container rejects >2 sync waits per instruction.
    Hoist excess waits onto standalone EventSemaphore insts on the same engine."""
    ctr = 0
    for fn in nc.m.functions:
        for bb in fn.blocks:
            out = []
            changed = False
            for inst in list(bb.instructions):
                si = inst.sync_info
                if si is not None and si.on_wait and len(si.on_wait) > max_waits:
                    extra = list(si.on_wait[:-max_waits])
                    keep = list(si.on_wait[-max_waits:])
                    for w in extra:
                        ctr += 1
                        ev = mybir.InstEventSemaphore(
                            name=f"I-waitsplit-{ctr}", ins=[], outs=[]
                        )
                        ev.engine = inst.engine
                        ev.sync_info = mybir.SyncInfo(on_wait=[w], on_update=[])
                        out.append(ev)
                    si.on_wait = keep
                    changed = True
                out.append(inst)
            if changed:
                bb.instructions = out
    return ctr



def _emit_softmax_and_toeplitz(nc, tc, stp, zmax, zraw, nbias, ez, zsum, rz, znR,
                               zpad, zero128, a_sb):
    nc.vector.reduce_max(zmax[:], zraw[:], axis=mybir.AxisListType.X)
    nc.scalar.mul(nbias[:], zmax[:], -SCALING)
    nc.scalar.activation(
        ez[:], zraw[:], mybir.ActivationFunctionType.Exp,
        bias=nbias[:], scale=SCALING, accum_out=zsum[:],
    )
    nc.vector.reciprocal(rz[:], zsum[:])
    # normalize + reverse in one DVE op: znR[h,m] = zn[h, 2047-m]
    nc.vector.tensor_scalar_mul(znR[:], ez[:, ::-1], rz[:])

    # zpad row h holds zn REVERSED (zpad[h,m] = zn_ext[2047-m]) with a
    # zero tail at [2048:2176] (implements the causal mask for L=0).
    nc.sync.dma_start(zpad[:, 0:S], znR[:])
    nc.sync.dma_start(zpad[:, S:ZW], zero128[:].bitcast(F32R))

    # stage_h[j, m] = zpad[h, j + m]  (sliding window, one fat DMA)
    # Toeplitz: A_L^T[j,i] = zn_ext[L*128+i-j] = stage_h[j, 2047-(L*128+i)],
    # so the full reversed stage IS the 16 lag-tiles concatenated: one
    # whole-row reversed DVE copy per head materializes all of them.
    for h in range(HPC):
        stage = stp.tile([128, S], F32R, tag="stage", name=f"stage{h}")
        nc.scalar.dma_start(stage[:], AP(zpad, h * ZW, [[1, 128], [1, S]]))
        nc.vector.tensor_copy(
            a_sb[:, h * S : (h + 1) * S], stage[:, ::-1]
        )


DEFAULT_SPEC = (("conv", 1), ("fin", 1), ("trans", 1), ("v", 1), ("z", 1))


def _build_nc(spec=DEFAULT_SPEC):
    reps = dict(spec)
    diag = any(r > 1 for r in reps.values())
    nc = bass.Bass()
    xT = nc.dram_tensor("xT", [E, S], F32R, kind="ExternalInput")
    # weights arrive host-pre-arranged in the exact SBUF layout (contiguous DMA)
    wat = nc.dram_tensor("wat", [128, KE * HPC], F32R, kind="ExternalInput")
    wvt = nc.dram_tensor("wvt", [128, KE * CB], F32R, kind="ExternalInput")
    wot = nc.dram_tensor("wot", [128, 2 * E], F32R, kind="ExternalInput")
    outp = nc.dram_tensor(
        "outp", [S, E], BF16 if OUT_BF16 else F32, kind="ExternalOutput"
    )
    zpad = nc.dram_tensor("zpad", [HPC, ZW], F32R)

    with tile.TileContext(nc) as tc:
        with (
            tc.tile_pool(name="per", bufs=1) as per,
            tc.tile_pool(name="fs", bufs=2) as fsp,
            tc.tile_pool(name="stp", bufs=(1 if diag else 2)) as stp,
        ):
            ident = per.tile([128, 128], F32, tag="ident")
            masks.make_identity(nc, ident[:])

            xTs = []
            for k in range(KE):
                t = per.tile([128, S], F32R, tag=f"xT{k}", name=f"xTsb{k}")
                xTs.append(t)
            # quartered loads so z/v matmuls can start before the full 8MB lands
            for q in range(4):
                for k in range(KE):
                    nc.sync.dma_start(
                        xTs[k][:, q * 512 : (q + 1) * 512],
                        xT[k * 128 : (k + 1) * 128, q * 512 : (q + 1) * 512],
                    )
            wat_sb = per.tile([128, KE * HPC], F32R, tag="wat")
            nc.sync.dma_start(wat_sb[:], wat[:])
            wvt_sb = per.tile([128, KE * CB], F32R, tag="wvt")
            nc.sync.dma_start(wvt_sb[:], wvt[:])
            wot_sb = per.tile([128, 2 * E], F32R, tag="wot")
            nc.sync.dma_start(wot_sb[:], wot[:])

            zraw = per.tile([HPC, S], F32, tag="zbig")
            ez = per.tile([HPC, S], F32, tag="ez")
            znR = per.tile(
                [HPC, S], F32R, tag=("znR" if diag else "zbig"), name="znR"
            )
            zero128 = per.tile([HPC, 128], F32, tag="zero")
            nc.vector.memset(zero128[:], 0.0)
            zmax = per.tile([HPC, 1], F32, tag="zmax")
            nbias = per.tile([HPC, 1], F32, tag="nbias")
            zsum = per.tile([HPC, 1], F32, tag="zsum")
            rz = per.tile([HPC, 1], F32, tag="rz")

            v_sb = per.tile([128, NB * CB], F32R, tag="v")
            o_sb = per.tile([128, NB * CB], F32, tag="o")
            oTs = [per.tile([128, S], F32R, tag=f"oT{g2}", name=f"oT{g2}") for g2 in range(2)]
            a_sb = per.tile([128, HPC * 16 * 128], F32R, tag="a")

            with (
                tc.tile_pool(name="zp", bufs=2, space="PSUM") as zpool,
                tc.tile_pool(name="vp", bufs=3, space="PSUM") as vpool,
            ):
                # z projection: z^T[h, s] accumulated over 8 e-chunks
                for n in range(4 * reps.get("z", 0)):
                    n = n % 4
                    zp = zpool.tile([HPC, 512], F32, tag="zp")
                    for k in range(KE):
                        nc.tensor.matmul(
                            zp[:],
                            wat_sb[:, k * HPC : (k + 1) * HPC],
                            xTs[k][:, n * 512 : (n + 1) * 512],
                            start=(k == 0),
                            stop=(k == KE - 1),
                        )
                    nc.vector.tensor_copy(zraw[:, n * 512 : (n + 1) * 512], zp[:])

                # softmax over s (free dim): exp(SCALING*(z - max)) / sum
                for _r in range(reps.get("z", 0)):
                    _emit_softmax_and_toeplitz(
                        nc, tc, stp, zmax, zraw, nbias, ez, zsum, rz, znR,
                        zpad, zero128, a_sb)

                # v projection: v[s, c] per seq block, accumulated over e-chunks
                for J in range(NB * reps.get("v", 0)):
                    J = J % NB
                    vp = vpool.tile([128, CB], F32, tag="vp")
                    for k in range(KE):
                        nc.tensor.matmul(
                            vp[:],
                            xTs[k][:, J * 128 : (J + 1) * 128],
                            wvt_sb[:, k * CB : (k + 1) * CB],
                            start=(k == 0),
                            stop=(k == KE - 1),
                        )
                    nc.vector.tensor_copy(v_sb[:, J * CB : (J + 1) * CB], vp[:])

            v3 = v_sb[:].rearrange("p (j c) -> p j c", c=CB)
            o3 = o_sb[:].rearrange("p (i c) -> p i c", c=CB)
            with (
                tc.tile_pool(name="op", bufs=2, space="PSUM") as opool,
                tc.tile_pool(name="tp", bufs=2, space="PSUM") as tpool,
                tc.tile_pool(name="fp", bufs=2, space="PSUM") as fpool,
            ):
                # causal Toeplitz matmul: out_I = sum_L A_L @ V_{I-L}
                # psum cols (I,c); bank0 = out blocks 0..7, bank1 = 8..15
                for h in range(HPC * reps.get("conv", 0)):
                    h = h % HPC
                    op = opool.tile([128, NB * 64], F32, tag="op")
                    for L in range(16):
                        aT = a_sb[
                            :, (h * 16 + L) * 128 : (h * 16 + L + 1) * 128
                        ]
                        n1 = 8 - L
                        if n1 > 0:
                            rhs = v3[:, 0:n1, h * 64 : (h + 1) * 64]
                            nc.tensor.matmul(
                                op[:, L * 64 : 512],
                                aT,
                                rhs,
                                start=(L == 0),
                                stop=(L == 7),
                                skip_group_check=True,
                            )
                        j0 = max(0, 8 - L)
                        rhs = v3[:, j0 : 16 - L, h * 64 : (h + 1) * 64]
                        nc.tensor.matmul(
                            op[:, max(8, L) * 64 : 1024],
                            aT,
                            rhs,
                            start=(L == 0),
                            stop=(L == 15),
                            skip_group_check=True,
                        )
                    nc.vector.tensor_copy(
                        o3[:, :, h * 64 : (h + 1) * 64],
                        op[:].rearrange("p (i c) -> p i c", c=64),
                    )

                # transpose out -> out^T (per 128-channel group) for final proj
                for g2 in range(2 * reps.get("trans", 0)):
                    g2 = g2 % 2
                    for I in range(NB):
                        tp = tpool.tile([128, 128], F32, tag="tp")
                        nc.tensor.transpose(
                            tp[:],
                            o_sb[:, I * CB + g2 * 128 : I * CB + (g2 + 1) * 128],
                            ident[:],
                        )
                        nc.vector.tensor_copy(oTs[g2][:, I * 128 : (I + 1) * 128], tp[:])

                # partial output projection: fin[s, f] = sum_c oT[c, s] wot[c, f]
                for J in range(NB * reps.get("fin", 0)):
                    J = J % NB
                    fs = fsp.tile([128, E], BF16 if OUT_BF16 else F32, tag="fs")
                    for half in range(2):
                        fp = fpool.tile([128, 512], F32, tag="fp")
                        for cc in range(2):
                            nc.tensor.matmul(
                                fp[:],
                                oTs[cc][:, J * 128 : (J + 1) * 128],
                                wot_sb[
                                    :, cc * E + half * 512 : cc * E + (half + 1) * 512
                                ],
                                start=(cc == 0),
                                stop=(cc == 1),
                            )
                        nc.vector.tensor_copy(
                            fs[:, half * 512 : (half + 1) * 512], fp[:]
                        )
                    nc.sync.dma_start(outp[J * 128 : (J + 1) * 128, :], fs[:])

    _split_excess_waits(nc)
    return nc


class _Runner:
    """Builds the Bass module once and keeps the jitted shard_map executable."""

    def __init__(self, spec=DEFAULT_SPEC):
        import jax
        from jax.sharding import Mesh, PartitionSpec

        try:
            from jax.experimental.shard_map import shard_map
        except ImportError:
            from jax.shard_map import shard_map

        from concourse import bass2jax

        bass2jax.install_neuronx_cc_hook()
        self.jax = jax
        nc = _build_nc(spec)
        self.nc = nc

        partition_name = (
            nc.partition_id_tensor.name if nc.partition_id_tensor else None
        )
        in_names, out_names, out_avals, zero_outs = [], [], [], []
        for alloc in nc.m.functions[0].allocations:
            if not isinstance(alloc, mybir.MemoryLocationSet):
                continue
            name = alloc.memorylocations[0].name
            if alloc.kind == "ExternalInput":
                if name != partition_name:
                    in_names.append(name)
            elif alloc.kind == "ExternalOutput":
                shape = tuple(alloc.tensor_shape)
                dtype = mybir.dt.np(alloc.dtype)
                out_names.append(name)
                out_avals.append(jax.core.ShapedArray(shape, dtype))
                zero_outs.append(np.zeros(shape, dtype))
        self.in_names = in_names
        self.out_names = out_names
        self.out_shapes = [tuple(a.shape) for a in out_avals]
        self.zero_outs = zero_outs
        n_params = len(in_names)
        n_outs = len(out_names)
        all_in_names = list(in_names) + list(out_names)
        if partition_name is not None:
            all_in_names.append(partition_name)

        def _body(*args):
            operands = list(args)
            if partition_name is not None:
                operands.append(bass2jax.partition_id_tensor())
            outs = bass2jax._bass_exec_p.bind(
                *operands,
                out_avals=tuple(out_avals),
                in_names=tuple(all_in_names),
                out_names=tuple(out_names),
                lowering_input_output_aliases=(),
                sim_require_finite=True,
                sim_require_nnan=True,
                nc=nc,
            )
            return tuple(outs)

        devices = jax.devices()[:NCORES]
        assert len(devices) == NCORES, f"need {NCORES} cores, got {len(devices)}"
        self.mesh = Mesh(np.asarray(devices), ("core",))
        in_specs = (PartitionSpec("core"),) * (n_params + n_outs)
        out_specs = (PartitionSpec("core"),) * n_outs
        donate = tuple(range(n_params, n_params + n_outs))
        self.sharded = jax.jit(
            shard_map(
                _body,
                mesh=self.mesh,
                in_specs=in_specs,
                out_specs=out_specs,
                check_rep=False,
            ),
            donate_argnums=donate,
            keep_unused=True,
        )
        # Non-donating variant for benchmarking: one zeros set can be reused
        # across dispatches (kernel writes every output element).
        self.sharded_nodonate = jax.jit(
            shard_map(
                _body,
                mesh=self.mesh,
                in_specs=in_specs,
                out_specs=out_specs,
                check_rep=False,
            ),
            keep_unused=True,
        )

    def concat_inputs(self, in_maps):
        return [
            np.concatenate([np.asarray(in_maps[c][nm]) for c in range(NCORES)], axis=0)
            for nm in self.in_names
        ]

    def fresh_zeros(self):
        return [
            np.zeros((NCORES * z.shape[0], *z.shape[1:]), z.dtype)
            for z in self.zero_outs
        ]

    def run_concat(self, concat_in, zeros):
        out_arrs = self.sharded(*concat_in, *zeros)
        return out_arrs

    def run(self, in_maps):
        out_arrs = self.run_concat(self.concat_inputs(in_maps), self.fresh_zeros())
        res = []
        for c in range(NCORES):
            res.append(
                {
                    nm: np.asarray(out_arrs[i]).reshape(
                        NCORES, *self.out_shapes[i]
                    )[c]
                    for i, nm in enumerate(self.out_names)
                }
            )
        return res


_RUNNERS = {}


def _get_runner(spec=DEFAULT_SPEC):
    spec = tuple(sorted(dict(spec).items()))
    if spec not in _RUNNERS:
        _RUNNERS[spec] = _Runner(spec)
    return _RUNNERS[spec]


def _shard_inputs(x, W_A, W_V, W_O):
    x = np.asarray(x, dtype=np.float32)
    W_A = np.asarray(W_A, dtype=np.float32)
    W_V = np.asarray(W_V, dtype=np.float32)
    W_O = np.asarray(W_O, dtype=np.float32)
    xTs = [np.ascontiguousarray(x[b].T) for b in range(B)]

    def sb_layout(wT, nk):
        # [nk*128, c] -> [128, nk*c]: partition p holds chunk-k cols at k*c
        c = wT.shape[1]
        return np.ascontiguousarray(
            wT.reshape(nk, 128, c).transpose(1, 0, 2).reshape(128, nk * c)
        )

    in_maps = []
    for c in range(NCORES):
        b, g = divmod(c, NCORES // B)
        r0, r1 = g * CB, (g + 1) * CB
        in_maps.append(
            {
                "xT": xTs[b],
                "wat": sb_layout(W_A[g * HPC : (g + 1) * HPC, :].T, KE),
                "wvt": sb_layout(W_V[r0:r1, :].T, KE),
                "wot": sb_layout(W_O[:, r0:r1].T, 2),
            }
        )
    return in_maps


def kernel(x, W_A, W_V, W_O, b_O):
    runner = _get_runner()
    in_maps = _shard_inputs(x, W_A, W_V, W_O)
    res = runner.run(in_maps)
    b_O = np.asarray(b_O, dtype=np.float32)
    out = np.empty((B, S, E), np.float32)
    gpb = NCORES // B
    for b in range(B):
        acc = res[b * gpb]["outp"].astype(np.float32)
        for g in range(1, gpb):
            acc = acc + res[b * gpb + g]["outp"].astype(np.float32)
        out[b] = acc + b_O
    return out


def _marginal_once(runner, dev_in, zset, k_small=4, k_big=64):
    import time

    def run_k(k):
        t0 = time.perf_counter()
        outs = None
        for _ in range(k):
            outs = runner.sharded_nodonate(*dev_in, *zset)
        for a in outs:
            a.block_until_ready()
        return time.perf_counter() - t0

    t_small = run_k(k_small)
    t_big = run_k(k_big)
    return (t_big - t_small) / (k_big - k_small) * 1e6


def measure_exec_ns(x, W_A, W_V, W_O, b_O, amp=17, pairs=7):
    """Per-execution device time: interleaved paired marginals of the normal
    kernel vs an `amp`-times-repeated body (drift-cancelling)."""
    import jax
    from jax.sharding import NamedSharding, PartitionSpec

    in_maps = _shard_inputs(x, W_A, W_V, W_O)
    setups = {}
    for factor in (1, amp):
        spec = tuple((p, factor) for p in ("z", "v", "conv", "fin", "trans"))
        runner = _get_runner(spec)
        sh = NamedSharding(runner.mesh, PartitionSpec("core"))
        dev_in = [jax.device_put(a, sh) for a in runner.concat_inputs(in_maps)]
        zset = [jax.device_put(z, sh) for z in runner.fresh_zeros()]
        for a in zset:
            a.block_until_ready()
        # warm
        _marginal_once(runner, dev_in, zset, 1, 2)
        setups[factor] = (runner, dev_in, zset)
    diffs = []
    m1s, mAs = [], []
    for _ in range(pairs):
        m1 = _marginal_once(*setups[1])
        mA = _marginal_once(*setups[amp])
        m1s.append(m1)
        mAs.append(mA)
        diffs.append((mA - m1) / (amp - 1))
    diffs.sort()
    med = diffs[len(diffs) // 2]
    return {
        "m1_us": [round(v) for v in m1s],
        f"m{amp}_us": [round(v) for v in mAs],
        "diffs_us": [round(v, 1) for v in sorted(diffs)],
        "per_exec_ns": int(med * 1e3),
    }



# revision 25
# speedup vs baseline: 1.2122x; 1.2122x over previous
"""CATAttention Trainium2 kernel (v2: all-bf16 data path).

Math: out[b,i,h,:] = sum_{j<=i} softmax_s(x@W_A^T)[b,i-j,h] * v[b,j,h,:]
i.e. a causal convolution along the sequence with a per-(b,h) data-dependent
kernel z. The [B,H,S,S] "roll" matrix is block-Toeplitz: its 128x128 blocks
depend only on the block lag L = I-J, so only 16 distinct blocks per head are
ever materialized. They are built in SBUF by a single sliding-window DMA per
head from a zero-headed copy of z in DRAM (partition stride -1); the 128-zero
head implements the causal mask for L=0.

Sharding (8 cores): core c -> batch b = c//4, head group g = c%4 (4 heads).
Each core computes z, v = x@W_V^T (its 256 channels), the causal Toeplitz
matmul, and a partial output projection against its 256 columns of W_O.
Host gathers: out[b] = sum of the 4 partials + b_O.

All tensors on the wire and all matmuls are bf16 (1 PE cycle/row at any
moving size); softmax runs in fp32 on the Activation engine without max
subtraction (logits are bounded by construction). PSUM evictions are spread
across DVE / Activation / Pool so they hide under the PE-bound phases.
"""

import numpy as np

import concourse.bass as bass
import concourse.mybir as mybir
import concourse.tile as tile
from concourse import masks
from concourse.ap import AP

F32 = mybir.dt.float32
BF16 = mybir.dt.bfloat16
NPBF16 = mybir.dt.np(mybir.dt.bfloat16)

B, S, E, H, D = 2, 2048, 1024, 16, 64
SCALING = D ** -0.5
NCORES = 8
HPC = 4            # heads per core
CB = HPC * D       # 256 channels per core
NB = S // 128      # 16 seq blocks
KE = E // 128      # 8 contraction chunks
ZW = 2176          # zfwd row width: 128 zeros + 2048 weights
WAT_OFF = 0        # wcat column offsets
WVT_OFF = KE * HPC            # 32
WOT_OFF = WVT_OFF + KE * CB   # 2080
WCOLS = WOT_OFF + 2 * E       # 4128


def _split_excess_waits(nc, max_waits=1):
    """The walrus in this container rejects >2 sync waits per instruction.
    Hoist excess waits onto standalone EventSemaphore insts on the same engine."""
    ctr = 0
    for fn in nc.m.functions:
        for bb in fn.blocks:
            out = []
            changed = False
            for inst in list(bb.instructions):
                si = inst.sync_info
                if si is not None and si.on_wait and len(si.on_wait) > max_waits:
                    extra = list(si.on_wait[:-max_waits])
                    keep = list(si.on_wait[-max_waits:])
                    for w in extra:
                        ctr += 1
                        ev = mybir.InstEventSemaphore(
                            name=f"I-waitsplit-{ctr}", ins=[], outs=[]
                        )
                        ev.engine = inst.engine
                        ev.sync_info = mybir.SyncInfo(on_wait=[w], on_update=[])
                        out.append(ev)
                    si.on_wait = keep
                    changed = True
                out.append(inst)
            if changed:
                bb.instructions = out
    return ctr


DEFAULT_SPEC = (("conv", 1), ("fin", 1), ("trans", 1), ("v", 1), ("z", 1))


def _build_nc(spec=DEFAULT_SPEC):
    reps = dict(spec)
    nc = bass.Bass()
    xq = nc.dram_tensor("xq", [128, 4 * KE * 512], BF16, kind="ExternalInput")
    wcat = nc.dram_tensor("wcat", [128, WCOLS], BF16, kind="ExternalInput")
    outp = nc.dram_tensor("outp", [S, E], BF16, kind="ExternalOutput")
    zfwd = nc.dram_tensor("zfwd", [HPC, ZW], BF16)
    rzd = nc.dram_tensor("rzd", [HPC, 1], F32)

    with tile.TileContext(nc) as tc:
        with (
            tc.tile_pool(name="per", bufs=1) as per,
            tc.tile_pool(name="fs", bufs=6) as fsp,
            tc.tile_pool(name="stp", bufs=4) as stp,
        ):
            ident = per.tile([128, 128], BF16, tag="ident")
            masks.make_identity(nc, ident[:])

            # weight loads ordered by first use: wat (z proj) first, wot
            # (final proj) last so the x quarters can slot in between.
            wcat_sb = per.tile([128, WCOLS], BF16, tag="wcat")
            nc.scalar.dma_start(
                wcat_sb[:, WAT_OFF:WVT_OFF], wcat[:, WAT_OFF:WVT_OFF]
            )

            zero128 = per.tile([HPC, 128], BF16, tag="zero")
            nc.vector.memset(zero128[:], 0.0)
            # causal-mask zero tail of the z scratch row (static content)
            nc.scalar.dma_start(zfwd[:, S:ZW], zero128[:])

            xq_sb = per.tile([128, 4 * KE * 512], BF16, tag="xq")
            # first half-quarter ASAP so the z matmuls can begin
            nc.sync.dma_start(xq_sb[:, 0 : 4 * 512], xq[:, 0 : 4 * 512])
            nc.scalar.dma_start(
                wcat_sb[:, WVT_OFF:WOT_OFF], wcat[:, WVT_OFF:WOT_OFF]
            )
            nc.sync.dma_start(xq_sb[:, 4 * 512 : 8 * 512], xq[:, 4 * 512 : 8 * 512])
            for q in range(1, 4):
                nc.sync.dma_start(
                    xq_sb[:, q * KE * 512 : (q + 1) * KE * 512],
                    xq[:, q * KE * 512 : (q + 1) * KE * 512],
                )
            nc.scalar.dma_start(
                wcat_sb[:, WOT_OFF:WCOLS], wcat[:, WOT_OFF:WCOLS]
            )

            zraw = per.tile([HPC, S], F32, tag="zraw")
            ez = per.tile([HPC, S], BF16, tag="ez")
            zsums = per.tile([HPC, 4], F32, tag="zsums")
            zsum = per.tile([HPC, 1], F32, tag="zsum")
            rz = per.tile([HPC, 1], F32, tag="rz")
            rz128 = [
                per.tile([128, 1], F32, tag=f"rz128_{g2}", name=f"rz128_{g2}")
                for g2 in range(2)
            ]

            v_sb = per.tile([128, NB * CB], BF16, tag="v")
            o_sb = per.tile([128, NB * CB], BF16, tag="o")
            oTs = [
                per.tile([128, S], BF16, tag=f"oT{g2}", name=f"oT{g2}")
                for g2 in range(2)
            ]
            a_sb = per.tile([128, HPC * S], BF16, tag="a")

            with (
                tc.tile_pool(name="zp", bufs=2, space="PSUM") as zpool,
                tc.tile_pool(name="vp", bufs=3, space="PSUM") as vpool,
            ):
                # z + v projections interleaved per x quarter.
                # z^T[h, s] and v[s, c] both contract over the 8 e-chunks.
                for q in range(4 * reps.get("v", 0)):
                    q = q % 4
                    zp = zpool.tile([HPC, 512], F32, tag="zp")
                    for k in range(KE):
                        nc.tensor.matmul(
                            zp[:],
                            wcat_sb[:, WAT_OFF + k * HPC : WAT_OFF + (k + 1) * HPC],
                            xq_sb[:, (q * KE + k) * 512 : (q * KE + k + 1) * 512],
                            start=(k == 0),
                            stop=(k == KE - 1),
                        )
                    nc.vector.tensor_copy(zraw[:, q * 512 : (q + 1) * 512], zp[:])
                    # exp per quarter on the Activation engine (no max
                    # subtraction: logits are ~N(0,1)/8, exp never overflows).
                    # The UNNORMALIZED exp is written REVERSED (ez[m] =
                    # e^z[2047-m]) and goes straight to DRAM so the Toeplitz
                    # sliding-window reads can start during the v phase;
                    # 1/sum is folded into the transpose evictions.
                    nc.scalar.activation(
                        ez[:, S - (q + 1) * 512 : S - q * 512][:, ::-1],
                        zraw[:, q * 512 : (q + 1) * 512],
                        mybir.ActivationFunctionType.Exp,
                        scale=SCALING,
                        accum_out=zsums[:, q : q + 1],
                    )
                    nc.scalar.dma_start(
                        zfwd[:, S - (q + 1) * 512 : S - q * 512],
                        ez[:, S - (q + 1) * 512 : S - q * 512],
                    )
                    for J in range(4 * q, 4 * q + 4):
                        vp = vpool.tile([128, CB], F32, tag="vp")
                        for k in range(KE):
                            nc.tensor.matmul(
                                vp[:],
                                xq_sb[
                                    :,
                                    (q * KE + k) * 512
                                    + (J % 4) * 128 : (q * KE + k) * 512
                                    + (J % 4 + 1) * 128,
                                ],
                                wcat_sb[:, WVT_OFF + k * CB : WVT_OFF + (k + 1) * CB],
                                start=(k == 0),
                                stop=(k == KE - 1),
                            )
                        nc.vector.tensor_copy(v_sb[:, J * CB : (J + 1) * CB], vp[:])

                # Toeplitz build: a_sb[j, h*S + f] = zn[f - j] (0 for f < j).
                # zfwd holds zn reversed with a zero tail, so the sliding
                # window stage[j, t'] = zfwd[h, base + j + t'] uses legal +1
                # strides; a cheap bf16 reversed DVE copy un-reverses the
                # free dim.  Issued per (quarter, head) piece so each starts
                # as soon as its ez quarters hit DRAM — almost all of it
                # lands during the v phase, and conv h0's early lags can
                # start immediately.
                for _r in range(reps.get("z", 0)):
                    for qa in range(4):
                        for h in range(HPC):
                            stg = stp.tile([128, 512], BF16, tag="stg", name="stg")
                            eng = nc.sync if (qa * HPC + h) % 2 == 0 else nc.scalar
                            eng.dma_start(
                                stg[:],
                                AP(
                                    zfwd,
                                    h * ZW + (3 - qa) * 512,
                                    [[1, 128], [1, 512]],
                                ),
                            )
                            nc.vector.tensor_copy(
                                a_sb[
                                    :, h * S + qa * 512 : h * S + (qa + 1) * 512
                                ],
                                stg[:, ::-1],
                            )
                    # 1/sum(ez) per head, broadcast to the two 128-channel
                    # groups' partitions via a DRAM round trip (off the
                    # critical path; first consumed by the tp evictions).
                    nc.vector.reduce_sum(
                        zsum[:], zsums[:], axis=mybir.AxisListType.X
                    )
                    nc.vector.reciprocal(rz[:], zsum[:])
                    nc.scalar.dma_start(rzd[:], rz[:])
                    for g2 in range(2):
                        nc.scalar.dma_start(
                            rz128[g2][:],
                            AP(rzd, g2 * 2, [[1, 2], [0, 64], [0, 1]]),
                        )

            v3 = v_sb[:].rearrange("p (j c) -> p j c", c=CB)
            o3 = o_sb[:].rearrange("p (i c) -> p i c", c=CB)
            with tc.tile_pool(name="op", bufs=2, space="PSUM") as opool:
                # causal Toeplitz matmul: out_I = sum_L A_L @ V_{I-L}
                # psum cols (I,c); bank0 = out blocks 0..7, bank1 = 8..15
                for h in range(HPC * reps.get("conv", 0)):
                    h = h % HPC
                    op = opool.tile([128, NB * 64], F32, tag="op")
                    for L in range(16):
                        aT = a_sb[
                            :, (h * 16 + L) * 128 : (h * 16 + L + 1) * 128
                        ]
                        n1 = 8 - L
                        if n1 > 0:
                            rhs = v3[:, 0:n1, h * 64 : (h + 1) * 64]
                            nc.tensor.matmul(
                                op[:, L * 64 : 512],
                                aT,
                                rhs,
                                start=(L == 0),
                                stop=(L == 7),
                                skip_group_check=True,
                            )
                        j0 = max(0, 8 - L)
                        rhs = v3[:, j0 : 16 - L, h * 64 : (h + 1) * 64]
                        nc.tensor.matmul(
                            op[:, max(8, L) * 64 : 1024],
                            aT,
                            rhs,
                            start=(L == 0),
                            stop=(L == 15),
                            skip_group_check=True,
                        )
                    op3 = op[:].rearrange("p (i c) -> p i c", c=64)
                    nc.vector.tensor_copy(o3[:, 0:8, h * 64 : (h + 1) * 64], op3[:, 0:8])
                    nc.scalar.copy(o3[:, 8:16, h * 64 : (h + 1) * 64], op3[:, 8:16])

            with (
                tc.tile_pool(name="tp", bufs=2, space="PSUM") as tpool,
                tc.tile_pool(name="fp", bufs=3, space="PSUM") as fpool,
            ):
                # transpose out -> out^T (both 128-channel groups of an I2
                # pair at once), then the final projection for those blocks:
                # fin[s, f] = sum_c oT[c, s] wot[c, f].  Software-pipelined
                # so the tp eviction of pair n overlaps the fin of pair n-1.
                def emit_trans(I2):
                    for g2 in range(2):
                        tp = tpool.tile([128, 256], BF16, tag="tp", name="tp")
                        for u in range(2):
                            I = I2 + u
                            nc.tensor.transpose(
                                tp[:, u * 128 : (u + 1) * 128],
                                o_sb[:, I * CB + g2 * 128 : I * CB + (g2 + 1) * 128],
                                ident[:],
                            )
                        if g2 == 0:
                            nc.scalar.mul(
                                oTs[g2][:, I2 * 128 : (I2 + 2) * 128],
                                tp[:],
                                rz128[g2][:],
                            )
                        else:
                            nc.vector.tensor_scalar_mul(
                                oTs[g2][:, I2 * 128 : (I2 + 2) * 128],
                                tp[:],
                                rz128[g2][:],
                            )

                def emit_fin(J):
                    fs = fsp.tile([128, E], BF16, tag="fs", name="fs")
                    fp = fpool.tile([128, E], F32, tag="fp", name="fp")
                    for half in range(2):
                        for cc in range(2):
                            nc.tensor.matmul(
                                fp[:, half * 512 : (half + 1) * 512],
                                oTs[cc][:, J * 128 : (J + 1) * 128],
                                wcat_sb[
                                    :,
                                    WOT_OFF
                                    + cc * E
                                    + half * 512 : WOT_OFF
                                    + cc * E
                                    + (half + 1) * 512,
                                ],
                                start=(cc == 0),
                                stop=(cc == 1),
                                skip_group_check=True,
                            )
                    nc.vector.tensor_copy(fs[:, 0:512], fp[:, 0:512])
                    nc.scalar.copy(fs[:, 512:E], fp[:, 512:E])
                    nc.sync.dma_start(outp[J * 128 : (J + 1) * 128, :], fs[:])

                if all(r == 1 for r in reps.values()):
                    emit_trans(0)
                    for I2 in range(2, NB, 2):
                        emit_trans(I2)
                        emit_fin(I2 - 2)
                        emit_fin(I2 - 1)
                    emit_fin(NB - 2)
                    emit_fin(NB - 1)
                else:
                    for _ in range(reps.get("trans", 0)):
                        for I2 in range(0, NB, 2):
                            emit_trans(I2)
                    for J in range(NB * reps.get("fin", 0)):
                        emit_fin(J % NB)

    _split_excess_waits(nc)
    return nc


class _Runner:
    """Builds the Bass module once and keeps the jitted shard_map executable."""

    def __init__(self, spec=DEFAULT_SPEC):
        import jax
        from jax.sharding import Mesh, PartitionSpec

        try:
            from jax.experimental.shard_map import shard_map
        except ImportError:
            from jax.shard_map import shard_map

        from concourse import bass2jax

        bass2jax.install_neuronx_cc_hook()
        self.jax = jax
        nc = _build_nc(spec)
        self.nc = nc

        partition_name = (
            nc.partition_id_tensor.name if nc.partition_id_tensor else None
        )
        in_names, out_names, out_avals, zero_outs = [], [], [], []
        for alloc in nc.m.functions[0].allocations:
            if not isinstance(alloc, mybir.MemoryLocationSet):
                continue
            name = alloc.memorylocations[0].name
            if alloc.kind == "ExternalInput":
                if name != partition_name:
                    in_names.append(name)
            elif alloc.kind == "ExternalOutput":
                shape = tuple(alloc.tensor_shape)
                dtype = mybir.dt.np(alloc.dtype)
                out_names.append(name)
                out_avals.append(jax.core.ShapedArray(shape, dtype))
                zero_outs.append(np.zeros(shape, dtype))
        self.in_names = in_names
        self.out_names = out_names
        self.out_shapes = [tuple(a.shape) for a in out_avals]
        self.zero_outs = zero_outs
        n_params = len(in_names)
        n_outs = len(out_names)
        all_in_names = list(in_names) + list(out_names)
        if partition_name is not None:
            all_in_names.append(partition_name)

        def _body(*args):
            operands = list(args)
            if partition_name is not None:
                operands.append(bass2jax.partition_id_tensor())
            outs = bass2jax._bass_exec_p.bind(
                *operands,
                out_avals=tuple(out_avals),
                in_names=tuple(all_in_names),
                out_names=tuple(out_names),
                lowering_input_output_aliases=(),
                sim_require_finite=True,
                sim_require_nnan=True,
                nc=nc,
            )
            return tuple(outs)

        devices = jax.devices()[:NCORES]
        assert len(devices) == NCORES, f"need {NCORES} cores, got {len(devices)}"
        self.mesh = Mesh(np.asarray(devices), ("core",))
        in_specs = (PartitionSpec("core"),) * (n_params + n_outs)
        out_specs = (PartitionSpec("core"),) * n_outs
        donate = tuple(range(n_params, n_params + n_outs))
        self.sharded = jax.jit(
            shard_map(
                _body,
                mesh=self.mesh,
                in_specs=in_specs,
                out_specs=out_specs,
                check_rep=False,
            ),
            donate_argnums=donate,
            keep_unused=True,
        )
        # Non-donating variant for benchmarking: one zeros set can be reused
        # across dispatches (kernel writes every output element).
        self.sharded_nodonate = jax.jit(
            shard_map(
                _body,
                mesh=self.mesh,
                in_specs=in_specs,
                out_specs=out_specs,
                check_rep=False,
            ),
            keep_unused=True,
        )

    def concat_inputs(self, in_maps):
        return [
            np.concatenate([np.asarray(in_maps[c][nm]) for c in range(NCORES)], axis=0)
            for nm in self.in_names
        ]

    def fresh_zeros(self):
        return [
            np.zeros((NCORES * z.shape[0], *z.shape[1:]), z.dtype)
            for z in self.zero_outs
        ]

    def run_concat(self, concat_in, zeros):
        out_arrs = self.sharded(*concat_in, *zeros)
        return out_arrs

    def run(self, in_maps):
        out_arrs = self.run_concat(self.concat_inputs(in_maps), self.fresh_zeros())
        res = []
        for c in range(NCORES):
            res.append(
                {
                    nm: np.asarray(out_arrs[i]).reshape(
                        NCORES, *self.out_shapes[i]
                    )[c]
                    for i, nm in enumerate(self.out_names)
                }
            )
        return res


_RUNNERS = {}


def _get_runner(spec=DEFAULT_SPEC):
    spec = tuple(sorted(dict(spec).items()))
    if spec not in _RUNNERS:
        _RUNNERS[spec] = _Runner(spec)
    return _RUNNERS[spec]


def _shard_inputs(x, W_A, W_V, W_O):
    x = np.asarray(x, dtype=np.float32)
    W_A = np.asarray(W_A, dtype=np.float32)
    W_V = np.asarray(W_V, dtype=np.float32)
    W_O = np.asarray(W_O, dtype=np.float32)

    # xq free layout (q, k, c): partition p = e % 128 of chunk k,
    # col (q*8 + k)*512 + c holds x[b, q*512 + c, k*128 + p]
    xqs = []
    for b in range(B):
        xT = x[b].T  # [E, S]
        xqs.append(
            np.ascontiguousarray(
                xT.reshape(KE, 128, 4, 512).transpose(1, 2, 0, 3).reshape(128, -1)
            ).astype(NPBF16)
        )

    def sb_layout(wT, nk):
        # [nk*128, c] -> [128, nk*c]: partition p holds chunk-k cols at k*c
        c = wT.shape[1]
        return wT.reshape(nk, 128, c).transpose(1, 0, 2).reshape(128, nk * c)

    in_maps = []
    for c in range(NCORES):
        b, g = divmod(c, NCORES // B)
        r0, r1 = g * CB, (g + 1) * CB
        wcat = np.concatenate(
            [
                sb_layout(W_A[g * HPC : (g + 1) * HPC, :].T, KE),
                sb_layout(W_V[r0:r1, :].T, KE),
                sb_layout(W_O[:, r0:r1].T, 2),
            ],
            axis=1,
        ).astype(NPBF16)
        in_maps.append({"xq": xqs[b], "wcat": np.ascontiguousarray(wcat)})
    return in_maps


def kernel(x, W_A, W_V, W_O, b_O):
    runner = _get_runner()
    in_maps = _shard_inputs(x, W_A, W_V, W_O)
    res = runner.run(in_maps)
    b_O = np.asarray(b_O, dtype=np.float32)
    out = np.empty((B, S, E), np.float32)
    gpb = NCORES // B
    for b in range(B):
        acc = res[b * gpb]["outp"].astype(np.float32)
        for g in range(1, gpb):
            acc = acc + res[b * gpb + g]["outp"].astype(np.float32)
        out[b] = acc + b_O
    return out


def _marginal_once(runner, dev_in, zset, k_small=4, k_big=64):
    import time

    def run_k(k):
        t0 = time.perf_counter()
        outs = None
        for _ in range(k):
            outs = runner.sharded_nodonate(*dev_in, *zset)
        for a in outs:
            a.block_until_ready()
        return time.perf_counter() - t0

    t_small = run_k(k_small)
    t_big = run_k(k_big)
    return (t_big - t_small) / (k_big - k_small) * 1e6


def measure_exec_ns(x, W_A, W_V, W_O, b_O, amp=17, pairs=7):
    """Per-execution device time: interleaved paired marginals of the normal
    kernel vs an `amp`-times-repeated body (drift-cancelling)."""
    import jax
    from jax.sharding import NamedSharding, PartitionSpec

    in_maps = _shard_inputs(x, W_A, W_V, W_O)
    setups = {}
    for factor in (1, amp):
        spec = tuple((p, factor) for p in ("z", "v", "conv", "fin", "trans"))
        runner = _get_runner(spec)
        sh = NamedSharding(runner.mesh, PartitionSpec("core"))
        dev_in = [jax.device_put(a, sh) for a in runner.concat_inputs(in_maps)]
        zset = [jax.device_put(z, sh) for z in runner.fresh_zeros()]
        for a in zset:
            a.block_until_ready()
        # warm
        _marginal_once(runner, dev_in, zset, 1, 2)
        setups[factor] = (runner, dev_in, zset)
    diffs = []
    m1s, mAs = [], []
    for _ in range(pairs):
        m1 = _marginal_once(*setups[1])
        mA = _marginal_once(*setups[amp])
        m1s.append(m1)
        mAs.append(mA)
        diffs.append((mA - m1) / (amp - 1))
    diffs.sort()
    med = diffs[len(diffs) // 2]
    return {
        "m1_us": [round(v) for v in m1s],
        f"m{amp}_us": [round(v) for v in mAs],
        "diffs_us": [round(v, 1) for v in sorted(diffs)],
        "per_exec_ns": int(med * 1e3),
    }


# revision 32
# speedup vs baseline: 1.2798x; 1.0557x over previous
"""CATAttention Trainium2 kernel (v2: all-bf16 data path).

Math: out[b,i,h,:] = sum_{j<=i} softmax_s(x@W_A^T)[b,i-j,h] * v[b,j,h,:]
i.e. a causal convolution along the sequence with a per-(b,h) data-dependent
kernel z. The [B,H,S,S] "roll" matrix is block-Toeplitz: its 128x128 blocks
depend only on the block lag L = I-J, so only 16 distinct blocks per head are
ever materialized. They are built in SBUF by a single sliding-window DMA per
head from a zero-headed copy of z in DRAM (partition stride -1); the 128-zero
head implements the causal mask for L=0.

Sharding (8 cores): core c -> batch b = c//4, head group g = c%4 (4 heads).
Each core computes z, v = x@W_V^T (its 256 channels), the causal Toeplitz
matmul, and a partial output projection against its 256 columns of W_O.
Host gathers: out[b] = sum of the 4 partials + b_O.

All tensors on the wire and all matmuls are bf16 (1 PE cycle/row at any
moving size); softmax runs in fp32 on the Activation engine without max
subtraction (logits are bounded by construction). PSUM evictions are spread
across DVE / Activation / Pool so they hide under the PE-bound phases.
"""

import numpy as np

import concourse.bass as bass
import concourse.mybir as mybir
import concourse.tile as tile
from concourse import masks
from concourse.ap import AP

F32 = mybir.dt.float32
BF16 = mybir.dt.bfloat16
NPBF16 = mybir.dt.np(mybir.dt.bfloat16)

B, S, E, H, D = 2, 2048, 1024, 16, 64
SCALING = D ** -0.5
NCORES = 8
HPC = 4            # heads per core
CB = HPC * D       # 256 channels per core
NB = S // 128      # 16 seq blocks
KE = E // 128      # 8 contraction chunks
ZW = 2176          # zfwd row width: 2048 weights (reversed) + 128 zeros
CBZ = CB + HPC     # 260: per-chunk wcat block = 256 W_V cols + 4 W_A cols
WOT_OFF = KE * CBZ            # 2080
WCOLS = WOT_OFF + 2 * E       # 4128


def _split_excess_waits(nc, max_waits=1):
    """The walrus in this container rejects >2 sync waits per instruction.
    Hoist excess waits onto standalone EventSemaphore insts on the same engine."""
    ctr = 0
    for fn in nc.m.functions:
        for bb in fn.blocks:
            out = []
            changed = False
            for inst in list(bb.instructions):
                si = inst.sync_info
                if si is not None and si.on_wait and len(si.on_wait) > max_waits:
                    extra = list(si.on_wait[:-max_waits])
                    keep = list(si.on_wait[-max_waits:])
                    for w in extra:
                        ctr += 1
                        ev = mybir.InstEventSemaphore(
                            name=f"I-waitsplit-{ctr}", ins=[], outs=[]
                        )
                        ev.engine = inst.engine
                        ev.sync_info = mybir.SyncInfo(on_wait=[w], on_update=[])
                        out.append(ev)
                    si.on_wait = keep
                    changed = True
                out.append(inst)
            if changed:
                bb.instructions = out
    return ctr


DEFAULT_SPEC = (("conv", 1), ("fin", 1), ("trans", 1), ("v", 1), ("z", 1))


def _build_nc(spec=DEFAULT_SPEC):
    reps = dict(spec)
    nc = bass.Bass()
    xq = nc.dram_tensor("xq", [128, 4 * KE * 512], BF16, kind="ExternalInput")
    wcat = nc.dram_tensor("wcat", [128, WCOLS], BF16, kind="ExternalInput")
    outp = nc.dram_tensor("outp", [S, E], BF16, kind="ExternalOutput")
    zfwd = nc.dram_tensor("zfwd", [HPC, ZW], BF16)
    rzd = nc.dram_tensor("rzd", [HPC, 1], F32)

    with tile.TileContext(nc) as tc:
        with (
            tc.tile_pool(name="per", bufs=1) as per,
            tc.tile_pool(name="fs", bufs=6) as fsp,
            tc.tile_pool(name="stp", bufs=4) as stp,
        ):
            ident = per.tile([128, 128], BF16, tag="ident")
            masks.make_identity(nc, ident[:])

            # loads ordered by first use: the k0 pieces of the weights and
            # the first x quarter land first so PE can start within ~3us.
            wcat_sb = per.tile([128, WCOLS], BF16, tag="wcat")
            xq_sb = per.tile([128, 4 * KE * 512], BF16, tag="xq")
            nc.scalar.dma_start(wcat_sb[:, 0:CBZ], wcat[:, 0:CBZ])
            nc.sync.dma_start(xq_sb[:, 0:512], xq[:, 0:512])
            nc.scalar.dma_start(wcat_sb[:, CBZ:WOT_OFF], wcat[:, CBZ:WOT_OFF])
            nc.sync.dma_start(xq_sb[:, 512 : KE * 512], xq[:, 512 : KE * 512])
            for q in range(1, 4):
                nc.sync.dma_start(
                    xq_sb[:, q * KE * 512 : (q + 1) * KE * 512],
                    xq[:, q * KE * 512 : (q + 1) * KE * 512],
                )
            nc.sync.dma_start(
                wcat_sb[:, WOT_OFF:WCOLS], wcat[:, WOT_OFF:WCOLS]
            )

            zero128 = per.tile([HPC, 128], BF16, tag="zero")
            nc.vector.memset(zero128[:], 0.0)
            # causal-mask zero tail of the z scratch row (static content)
            nc.scalar.dma_start(zfwd[:, S:ZW], zero128[:])

            ez = per.tile([HPC, S], BF16, tag="ez")
            zsums = per.tile([HPC, 4], F32, tag="zsums")
            zsum = per.tile([HPC, 1], F32, tag="zsum")
            rz = per.tile([HPC, 1], F32, tag="rz")
            rz128 = [
                per.tile([128, 1], F32, tag=f"rz128_{g2}", name=f"rz128_{g2}")
                for g2 in range(2)
            ]

            v_sb = per.tile([128, NB * CBZ], BF16, tag="v")
            o_sb = per.tile([128, NB * CB], BF16, tag="o")
            oTs = [
                per.tile([128, S], BF16, tag=f"oT{g2}", name=f"oT{g2}")
                for g2 in range(2)
            ]
            a_sb = per.tile([128, HPC * S], BF16, tag="a")

            with (
                tc.tile_pool(name="zp", bufs=2, space="PSUM") as zpool,
                tc.tile_pool(name="vp", bufs=3, space="PSUM") as vpool,
            ):
                # Fused [v | z] projection: each (J, k) matmul reuses one
                # loaded x stationary for 256 W_V columns plus the 4 W_A
                # columns, so the z projection is nearly free.  The z block
                # lands in [s, h] layout; a PE transpose per J flips it to
                # [h, s] psum, from which the Activation engine computes the
                # REVERSED unnormalized exp (ez[m] = e^z[2047-m]) straight to
                # SBUF and DMA to DRAM, overlapping the remaining v work (no
                # max subtraction: logits are ~N(0,1)/8, exp never
                # overflows).  1/sum is folded into the transpose evictions.
                tzps = {}

                def emit_ztail(q):
                    tzp = tzps.pop(q)
                    nc.scalar.activation(
                        ez[:, S - (q + 1) * 512 : S - q * 512][:, ::-1],
                        tzp[:],
                        mybir.ActivationFunctionType.Exp,
                        scale=SCALING,
                        accum_out=zsums[:, q : q + 1],
                    )
                    nc.scalar.dma_start(
                        zfwd[:, S - (q + 1) * 512 : S - q * 512],
                        ez[:, S - (q + 1) * 512 : S - q * 512],
                    )

                def emit_ztrans(J):
                    q = J // 4
                    if q not in tzps:
                        tzps[q] = zpool.tile([HPC, 512], BF16, tag="tzp", name="tzp")
                    nc.tensor.transpose(
                        tzps[q][:, (J % 4) * 128 : (J % 4 + 1) * 128],
                        v_sb[:, J * CBZ + CB : (J + 1) * CBZ],
                        ident[:],
                    )
                    if J % 4 == 3:
                        emit_ztail(q)

                for J in range(NB * reps.get("v", 0)):
                    J = J % NB
                    q = J // 4
                    vp = vpool.tile([128, CBZ], F32, tag="vp")
                    for k in range(KE):
                        nc.tensor.matmul(
                            vp[:],
                            xq_sb[
                                :,
                                (q * KE + k) * 512
                                + (J % 4) * 128 : (q * KE + k) * 512
                                + (J % 4 + 1) * 128,
                            ],
                            wcat_sb[:, k * CBZ : (k + 1) * CBZ],
                            start=(k == 0),
                            stop=(k == KE - 1),
                        )
                    if J % 4 == 3:
                        # split eviction: the z columns first so the chain
                        # transpose -> exp -> DRAM isn't gated on the wide copy
                        nc.vector.tensor_copy(
                            v_sb[:, J * CBZ + CB : (J + 1) * CBZ], vp[:, CB:CBZ]
                        )
                        nc.vector.tensor_copy(
                            v_sb[:, J * CBZ : J * CBZ + CB], vp[:, 0:CB]
                        )
                    else:
                        nc.vector.tensor_copy(
                            v_sb[:, J * CBZ : (J + 1) * CBZ], vp[:]
                        )
                    if J > 0:
                        emit_ztrans(J - 1)
                    if J == NB - 1:
                        emit_ztrans(J)

                # Toeplitz build: a_sb[j, h*S + f] = zn[f - j] (0 for f < j).
                # zfwd holds zn reversed with a zero tail, so the sliding
                # window stage[j, t'] = zfwd[h, base + j + t'] uses legal +1
                # strides; a cheap bf16 reversed DVE copy un-reverses the
                # free dim.  Issued per (quarter, head) piece so each starts
                # as soon as its ez quarters hit DRAM — almost all of it
                # lands during the v phase, and conv h0's early lags can
                # start immediately.
                for _r in range(reps.get("z", 0)):
                    for qa in range(4):
                        for h in range(HPC):
                            stg = stp.tile([128, 512], BF16, tag="stg", name="stg")
                            eng = nc.sync if (qa * HPC + h) % 2 == 0 else nc.scalar
                            eng.dma_start(
                                stg[:],
                                AP(
                                    zfwd,
                                    h * ZW + (3 - qa) * 512,
                                    [[1, 128], [1, 512]],
                                ),
                            )
                            nc.vector.tensor_copy(
                                a_sb[
                                    :, h * S + qa * 512 : h * S + (qa + 1) * 512
                                ],
                                stg[:, ::-1],
                            )
                    # 1/sum(ez) per head, broadcast to the two 128-channel
                    # groups' partitions via a DRAM round trip (off the
                    # critical path; first consumed by the tp evictions).
                    nc.vector.reduce_sum(
                        zsum[:], zsums[:], axis=mybir.AxisListType.X
                    )
                    nc.vector.reciprocal(rz[:], zsum[:])
                    nc.scalar.dma_start(rzd[:], rz[:])
                    for g2 in range(2):
                        nc.scalar.dma_start(
                            rz128[g2][:],
                            AP(rzd, g2 * 2, [[1, 2], [0, 64], [0, 1]]),
                        )

            v3 = v_sb[:].rearrange("p (j c) -> p j c", c=CBZ)
            o3 = o_sb[:].rearrange("p (i c) -> p i c", c=CB)
            with tc.tile_pool(name="op", bufs=2, space="PSUM") as opool:
                # causal Toeplitz matmul: out_I = sum_L A_L @ V_{I-L}
                # psum cols (I,c); bank0 = out blocks 0..7, bank1 = 8..15
                for h in range(HPC * reps.get("conv", 0)):
                    h = h % HPC
                    op = opool.tile([128, NB * 64], F32, tag="op")
                    for L in range(16):
                        aT = a_sb[
                            :, (h * 16 + L) * 128 : (h * 16 + L + 1) * 128
                        ]
                        n1 = 8 - L
                        if n1 > 0:
                            rhs = v3[:, 0:n1, h * 64 : (h + 1) * 64]
                            nc.tensor.matmul(
                                op[:, L * 64 : 512],
                                aT,
                                rhs,
                                start=(L == 0),
                                stop=(L == 7),
                                skip_group_check=True,
                            )
                        j0 = max(0, 8 - L)
                        rhs = v3[:, j0 : 16 - L, h * 64 : (h + 1) * 64]
                        nc.tensor.matmul(
                            op[:, max(8, L) * 64 : 1024],
                            aT,
                            rhs,
                            start=(L == 0),
                            stop=(L == 15),
                            skip_group_check=True,
                        )
                    op3 = op[:].rearrange("p (i c) -> p i c", c=64)
                    nc.vector.tensor_copy(o3[:, 0:8, h * 64 : (h + 1) * 64], op3[:, 0:8])
                    nc.scalar.copy(o3[:, 8:16, h * 64 : (h + 1) * 64], op3[:, 8:16])

            with (
                tc.tile_pool(name="tp", bufs=2, space="PSUM") as tpool,
                tc.tile_pool(name="fp", bufs=3, space="PSUM") as fpool,
            ):
                # transpose out -> out^T (both 128-channel groups of an I2
                # pair at once), then the final projection for those blocks:
                # fin[s, f] = sum_c oT[c, s] wot[c, f].  Software-pipelined
                # so the tp eviction of pair n overlaps the fin of pair n-1.
                def emit_trans(I2):
                    for g2 in range(2):
                        tp = tpool.tile([128, 256], BF16, tag="tp", name="tp")
                        for u in range(2):
                            I = I2 + u
                            nc.tensor.transpose(
                                tp[:, u * 128 : (u + 1) * 128],
                                o_sb[:, I * CB + g2 * 128 : I * CB + (g2 + 1) * 128],
                                ident[:],
                            )
                        if g2 == 0:
                            nc.scalar.mul(
                                oTs[g2][:, I2 * 128 : (I2 + 2) * 128],
                                tp[:],
                                rz128[g2][:],
                            )
                        else:
                            nc.vector.tensor_scalar_mul(
                                oTs[g2][:, I2 * 128 : (I2 + 2) * 128],
                                tp[:],
                                rz128[g2][:],
                            )

                def emit_fin(J):
                    fs = fsp.tile([128, E], BF16, tag="fs", name="fs")
                    fp = fpool.tile([128, E], F32, tag="fp", name="fp")
                    for half in range(2):
                        for cc in range(2):
                            nc.tensor.matmul(
                                fp[:, half * 512 : (half + 1) * 512],
                                oTs[cc][:, J * 128 : (J + 1) * 128],
                                wcat_sb[
                                    :,
                                    WOT_OFF
                                    + cc * E
                                    + half * 512 : WOT_OFF
                                    + cc * E
                                    + (half + 1) * 512,
                                ],
                                start=(cc == 0),
                                stop=(cc == 1),
                                skip_group_check=True,
                            )
                    nc.vector.tensor_copy(fs[:, 0:512], fp[:, 0:512])
                    nc.scalar.copy(fs[:, 512:E], fp[:, 512:E])
                    nc.sync.dma_start(outp[J * 128 : (J + 1) * 128, :], fs[:])

                if all(r == 1 for r in reps.values()):
                    emit_trans(0)
                    for I2 in range(2, NB, 2):
                        emit_trans(I2)
                        emit_fin(I2 - 2)
                        emit_fin(I2 - 1)
                    emit_fin(NB - 2)
                    emit_fin(NB - 1)
                else:
                    for _ in range(reps.get("trans", 0)):
                        for I2 in range(0, NB, 2):
                            emit_trans(I2)
                    for J in range(NB * reps.get("fin", 0)):
                        emit_fin(J % NB)

    _split_excess_waits(nc)
    return nc


class _Runner:
    """Builds the Bass module once and keeps the jitted shard_map executable."""

    def __init__(self, spec=DEFAULT_SPEC):
        import jax
        from jax.sharding import Mesh, PartitionSpec

        try:
            from jax.experimental.shard_map import shard_map
        except ImportError:
            from jax.shard_map import shard_map

        from concourse import bass2jax

        bass2jax.install_neuronx_cc_hook()
        self.jax = jax
        nc = _build_nc(spec)
        self.nc = nc

        partition_name = (
            nc.partition_id_tensor.name if nc.partition_id_tensor else None
        )
        in_names, out_names, out_avals, zero_outs = [], [], [], []
        for alloc in nc.m.functions[0].allocations:
            if not isinstance(alloc, mybir.MemoryLocationSet):
                continue
            name = alloc.memorylocations[0].name
            if alloc.kind == "ExternalInput":
                if name != partition_name:
                    in_names.append(name)
            elif alloc.kind == "ExternalOutput":
                shape = tuple(alloc.tensor_shape)
                dtype = mybir.dt.np(alloc.dtype)
                out_names.append(name)
                out_avals.append(jax.core.ShapedArray(shape, dtype))
                zero_outs.append(np.zeros(shape, dtype))
        self.in_names = in_names
        self.out_names = out_names
        self.out_shapes = [tuple(a.shape) for a in out_avals]
        self.zero_outs = zero_outs
        n_params = len(in_names)
        n_outs = len(out_names)
        all_in_names = list(in_names) + list(out_names)
        if partition_name is not None:
            all_in_names.append(partition_name)

        def _body(*args):
            operands = list(args)
            if partition_name is not None:
                operands.append(bass2jax.partition_id_tensor())
            outs = bass2jax._bass_exec_p.bind(
                *operands,
                out_avals=tuple(out_avals),
                in_names=tuple(all_in_names),
                out_names=tuple(out_names),
                lowering_input_output_aliases=(),
                sim_require_finite=True,
                sim_require_nnan=True,
                nc=nc,
            )
            return tuple(outs)

        devices = jax.devices()[:NCORES]
        assert len(devices) == NCORES, f"need {NCORES} cores, got {len(devices)}"
        self.mesh = Mesh(np.asarray(devices), ("core",))
        in_specs = (PartitionSpec("core"),) * (n_params + n_outs)
        out_specs = (PartitionSpec("core"),) * n_outs
        donate = tuple(range(n_params, n_params + n_outs))
        self.sharded = jax.jit(
            shard_map(
                _body,
                mesh=self.mesh,
                in_specs=in_specs,
                out_specs=out_specs,
                check_rep=False,
            ),
            donate_argnums=donate,
            keep_unused=True,
        )
        # Non-donating variant for benchmarking: one zeros set can be reused
        # across dispatches (kernel writes every output element).
        self.sharded_nodonate = jax.jit(
            shard_map(
                _body,
                mesh=self.mesh,
                in_specs=in_specs,
                out_specs=out_specs,
                check_rep=False,
            ),
            keep_unused=True,
        )

    def concat_inputs(self, in_maps):
        return [
            np.concatenate([np.asarray(in_maps[c][nm]) for c in range(NCORES)], axis=0)
            for nm in self.in_names
        ]

    def fresh_zeros(self):
        return [
            np.zeros((NCORES * z.shape[0], *z.shape[1:]), z.dtype)
            for z in self.zero_outs
        ]

    def run_concat(self, concat_in, zeros):
        out_arrs = self.sharded(*concat_in, *zeros)
        return out_arrs

    def run(self, in_maps):
        out_arrs = self.run_concat(self.concat_inputs(in_maps), self.fresh_zeros())
        res = []
        for c in range(NCORES):
            res.append(
                {
                    nm: np.asarray(out_arrs[i]).reshape(
                        NCORES, *self.out_shapes[i]
                    )[c]
                    for i, nm in enumerate(self.out_names)
                }
            )
        return res


_RUNNERS = {}


def _get_runner(spec=DEFAULT_SPEC):
    spec = tuple(sorted(dict(spec).items()))
    if spec not in _RUNNERS:
        _RUNNERS[spec] = _Runner(spec)
    return _RUNNERS[spec]


def _shard_inputs(x, W_A, W_V, W_O):
    x = np.asarray(x, dtype=np.float32)
    W_A = np.asarray(W_A, dtype=np.float32)
    W_V = np.asarray(W_V, dtype=np.float32)
    W_O = np.asarray(W_O, dtype=np.float32)

    # xq free layout (q, k, c): partition p = e % 128 of chunk k,
    # col (q*8 + k)*512 + c holds x[b, q*512 + c, k*128 + p]
    xqs = []
    for b in range(B):
        xT = x[b].T  # [E, S]
        xqs.append(
            np.ascontiguousarray(
                xT.reshape(KE, 128, 4, 512).transpose(1, 2, 0, 3).reshape(128, -1)
            ).astype(NPBF16)
        )

    def sb_layout(wT, nk):
        # [nk*128, c] -> [128, nk, c]: partition p holds chunk-k cols at k*c
        c = wT.shape[1]
        return wT.reshape(nk, 128, c).transpose(1, 0, 2)

    in_maps = []
    for c in range(NCORES):
        b, g = divmod(c, NCORES // B)
        r0, r1 = g * CB, (g + 1) * CB
        # per chunk k: 256 W_V columns then 4 W_A columns (CBZ block)
        wvz = np.concatenate(
            [
                sb_layout(W_V[r0:r1, :].T, KE),
                sb_layout(W_A[g * HPC : (g + 1) * HPC, :].T, KE),
            ],
            axis=2,
        ).reshape(128, KE * CBZ)
        wcat = np.concatenate(
            [wvz, sb_layout(W_O[:, r0:r1].T, 2).reshape(128, 2 * E)],
            axis=1,
        ).astype(NPBF16)
        in_maps.append({"xq": xqs[b], "wcat": np.ascontiguousarray(wcat)})
    return in_maps


def kernel(x, W_A, W_V, W_O, b_O):
    runner = _get_runner()
    in_maps = _shard_inputs(x, W_A, W_V, W_O)
    res = runner.run(in_maps)
    b_O = np.asarray(b_O, dtype=np.float32)
    out = np.empty((B, S, E), np.float32)
    gpb = NCORES // B
    for b in range(B):
        acc = res[b * gpb]["outp"].astype(np.float32)
        for g in range(1, gpb):
            acc = acc + res[b * gpb + g]["outp"].astype(np.float32)
        out[b] = acc + b_O
    return out


def _marginal_once(runner, dev_in, zset, k_small=4, k_big=64):
    import time

    def run_k(k):
        t0 = time.perf_counter()
        outs = None
        for _ in range(k):
            outs = runner.sharded_nodonate(*dev_in, *zset)
        for a in outs:
            a.block_until_ready()
        return time.perf_counter() - t0

    t_small = run_k(k_small)
    t_big = run_k(k_big)
    return (t_big - t_small) / (k_big - k_small) * 1e6


def measure_exec_ns(x, W_A, W_V, W_O, b_O, amp=17, pairs=7):
    """Per-execution device time: interleaved paired marginals of the normal
    kernel vs an `amp`-times-repeated body (drift-cancelling)."""
    import jax
    from jax.sharding import NamedSharding, PartitionSpec

    in_maps = _shard_inputs(x, W_A, W_V, W_O)
    setups = {}
    for factor in (1, amp):
        spec = tuple((p, factor) for p in ("z", "v", "conv", "fin", "trans"))
        runner = _get_runner(spec)
        sh = NamedSharding(runner.mesh, PartitionSpec("core"))
        dev_in = [jax.device_put(a, sh) for a in runner.concat_inputs(in_maps)]
        zset = [jax.device_put(z, sh) for z in runner.fresh_zeros()]
        for a in zset:
            a.block_until_ready()
        # warm
        _marginal_once(runner, dev_in, zset, 1, 2)
        setups[factor] = (runner, dev_in, zset)
    diffs = []
    m1s, mAs = [], []
    for _ in range(pairs):
        m1 = _marginal_once(*setups[1])
        mA = _marginal_once(*setups[amp])
        m1s.append(m1)
        mAs.append(mA)
        diffs.append((mA - m1) / (amp - 1))
    diffs.sort()
    med = diffs[len(diffs) // 2]
    return {
        "m1_us": [round(v) for v in m1s],
        f"m{amp}_us": [round(v) for v in mAs],
        "diffs_us": [round(v, 1) for v in sorted(diffs)],
        "per_exec_ns": int(med * 1e3),
    }
